# revision 1
# baseline (speedup 1.0000x reference)
"""Trainium2 Bass kernel for nn_AdditionFFN (4-step byte-addition FFN).

Reference semantics: 4 sequential steps; step i forms x = [a_i, b_i, carry]
(len 514), takes softmax(10*(x @ W1 - 2.5)) over 131072 one-hot table
entries, then result_i = weights @ W2_sum and carry' = weights @ W2_carry.

The tables are the deterministic one-hot structures from the reference's
_build_tables() (entry idx = a*512 + b*2 + c scores
a_emb[i,a] + b_emb[i,b] + carry[c]); kernel() verifies this structure
exactly and falls back to a direct on-device jax evaluation if it ever
fails to hold.  Under that structure the softmax factorizes:

    exp-scores = (ea ⊗ eb) ⊗ [e^{c0}, e^{c1}],  ea/eb = exp(10*emb - 12.5)

so weights @ W2_sum collapses to a 256-point circular convolution
u = ea (*) eb plus a roll by one for the odd-parity (carry-in) half, and
the carry chain reduces to a scalar logistic recurrence:

    Z_i  = sum(u_i)  (= sum(ea)*sum(eb))
    p1_i = sum_{a+b>=256} ea[a] eb[b]          (carry-out mass)
    ga_i = sigmoid(s_i)/Z_i ;  s_{i+1} = A_i + Q_i*ga_i,  s_0 = -10
    A_i  = 20*p1_i/Z_i - 10 ;  Q_i = 20*u_i[255]
    out_i = (1/Z_i - ga_i)*u_i + ga_i*roll(u_i, 1)

Device implementation (SPMD on 8 NeuronCores, no collectives — the problem
is tiny after factorization, so every core computes the full answer
redundantly and core 0's output is returned):

  - circular conv as TensorE correlation: lhsT row q = ea[255-q]
    (block-diagonal bf16 [128, 4] chunks so all 4 steps accumulate into one
    psum [4, 256]); rhs windows W_i[q, d] = eb2[i, 1+q+d] come from ONE
    overlapping-run DMA of the host-cast fp16 [b, b] copy: since b2h is
    [4, 512] contiguous, the run flat[1+q : 1+q+1920] covers all four
    steps' windows at column 512*i (128 fat descriptors instead of 512
    thin ones; fp16 keeps worst-case output error ~2.7e-3 vs 2e-2 gate)
  - p1 via a constant strict-upper-triangular bf16 rhs R[q, b] = 1{b > q}
    and a fused multiply-accumulate dot with exp(b) in fp32
  - cross-partition stat moves via DVE 32x32 block transposes (stats staged
    in columns 0/32/64 so transposed rows all land on partition 0)
  - carry chain tracked as sigma_i = sigmoid(s_i): exp + immediate-add +
    reciprocal + fused multiply-add per step (B_i = Q_i/Z_i precomputed;
    step 0 is fully constant-folded since s_0 = -10 is compile-time)
  - ~3us of dummy matmuls during the window DMA keep the PE HAM clock-gate
    open so the real matmuls run at 2.4 GHz (131 ns vs 213 ns spacing)
"""

import sys

sys.path.insert(0, "/opt/trn_rl_repo")

import numpy as np

import concourse.bacc as bacc
import concourse.mybir as mybir
import concourse.tile as tile
from concourse.ap import AP
from concourse.bass_utils import run_bass_kernel_spmd

N_CORES = 8
D = 256
F32 = mybir.dt.float32
BF16 = mybir.dt.bfloat16
FP16 = mybir.dt.float16
EXP = mybir.ActivationFunctionType.Exp
COPY = mybir.ActivationFunctionType.Copy
MULT = mybir.AluOpType.mult
ADD = mybir.AluOpType.add
E10 = float(np.exp(10.0))


def build_nc():
    nc = bacc.Bacc(None, target_bir_lowering=False, debug=False,
                   enable_partition_id=False)

    arevT2 = nc.declare_dram_parameter("arevT2", [128, 8], F32, isOutput=False)
    b2h = nc.declare_dram_parameter("b2h", [4, 2 * D], FP16, isOutput=False)
    out = nc.declare_dram_parameter("out", [4, D], F32, isOutput=True)

    with tile.TileContext(nc) as tc:
        with (
            tc.tile_pool(name="pool", bufs=1) as pool,
            tc.tile_pool(name="psum", bufs=1, space="PSUM") as psum,
        ):
            bias128 = pool.tile([128, 1], F32, tag="bias128")
            nc.vector.memset(bias128[:], -12.5)
            bias4 = pool.tile([4, 1], F32, tag="bias4")
            nc.vector.memset(bias4[:], -12.5)
            # dummy exp so the ACT exp-table load overlaps the input DMAs
            dummy = pool.tile([128, 1], F32, tag="dummy")
            nc.scalar.activation(dummy[:], bias128[:], EXP, bias=bias128[:],
                                 scale=10.0)

            # --- lhsT source first on gpsimd (tiny, gates all matmuls)
            araw = pool.tile([128, 8], F32, tag="araw")
            nc.gpsimd.dma_start(araw[:], arevT2.ap())

            # --- ONE mega-window DMA: b2h is [4, 512] contiguous, so the run
            # flat[1+q : 1+q+1920] covers every step's window at col 512*i
            # (128 fat descriptors instead of 512 thin ones)
            Wall = pool.tile([128, 1920], FP16, tag="Wall")
            nc.sync.dma_start(Wall[0:64, :], AP(b2h, 1, [[1, 64], [1, 1920]]))
            nc.gpsimd.dma_start(Wall[64:128, :],
                                AP(b2h, 65, [[1, 64], [1, 1920]]))

            # --- constant triangular rhs built on-device:
            # trit[q, c*256 + b] = 1{b > 128c + q}
            ones = pool.tile([128, 2 * D], BF16, tag="ones")
            nc.vector.memset(ones[:], 1.0)

            # PE warmup: ~3.4us of dummy matmuls while the window DMA is in
            # flight, so the HAM clock-gate opens (1.2 -> 2.4 GHz) before the
            # real matmuls issue
            Pd = psum.tile([4, 2 * D], F32, tag="Pd")
            for j in range(7):
                nc.tensor.matmul(
                    Pd[:], ones[:, 0:4], ones[:],
                    start=(j == 0), stop=(j == 6),
                )

            trit = pool.tile([128, 2 * D], BF16, tag="trit")
            nc.gpsimd.affine_select(
                trit[:].rearrange("p (c b) -> p c b", c=2), 
                ones[:].rearrange("p (c b) -> p c b", c=2),
                [[-128, 2], [1, 256]], mybir.AluOpType.is_gt, 0.0,
                base=0, channel_multiplier=-1,
            )

            # --- lhsT: block-diagonal bf16 [128, 32] (exp'd before windows so
            # bd is ready as soon as the first window lands)
            ebuf = pool.tile([128, 8], BF16, tag="ebuf")
            nc.scalar.activation(ebuf[:], araw[:], EXP, bias=bias128[:],
                                 scale=10.0)
            bd = pool.tile([128, 32], BF16, tag="bd")
            nc.vector.memset(bd[:], 0.0)
            for c in range(2):
                nc.vector.tensor_copy(
                    bd[:, 4 * c:4 * c + 28:9], ebuf[:, 4 * c:4 * c + 4]
                )

            # --- strided exps over the used window slices, split in two so
            # steps 0-1's matmuls start while steps 2-3's exp still runs
            Wcat = pool.tile([128, 4 * 384], BF16, tag="Wcat")
            for h in range(2):
                wbase = Wall[:, 1024 * h:1024 * h + 1]
                nc.scalar.activation(
                    Wcat[:, 768 * h:768 * h + 768].rearrange(
                        "p (i f) -> p i f", i=2),
                    AP(wbase.tensor, wbase.offset,
                       [list(wbase.ap[0]), [512, 2], [1, 384]]),
                    EXP, bias=bias128[:], scale=10.0,
                )

            # --- eb row (fp32) for the p1 dot (only needed by the p1 stt)
            braw = pool.tile([4, D], FP16, tag="braw")
            nc.gpsimd.dma_start(braw[:], b2h.ap()[:, 0:D])
            ebrow = pool.tile([4, D], F32, tag="ebrow")
            nc.scalar.activation(ebrow[:], braw[:], EXP, bias=bias4[:],
                                 scale=10.0)

            # --- matmuls: P_u (windows, ready first) then P_w (tri lands later)
            P_w = psum.tile([4, D], F32, tag="P_w")
            P_u = psum.tile([4, D], F32, tag="P_u")
            for i in range(4):
                for c in range(2):
                    k = 2 * i + c
                    nc.tensor.matmul(
                        P_u[:], bd[:, 4 * k:4 * k + 4],
                        Wcat[:, 384 * i + 128 * c:384 * i + 128 * c + 256],
                        start=(k == 0), stop=(k == 7),
                    )
            for i in range(4):
                for c in range(2):
                    k = 2 * i + c
                    nc.tensor.matmul(
                        P_w[:], bd[:, 4 * k:4 * k + 4],
                        trit[:, c * D:(c + 1) * D],
                        start=(k == 0), stop=(k == 7),
                    )

            # --- stats into staging columns 0 (Z), 32 (p1), 64 (u255)
            stg1 = pool.tile([32, 96], F32, tag="stg1")
            nc.vector.memset(stg1[:], 0.0)
            nc.vector.reduce_sum(stg1[0:4, 0:1], P_u[:],
                                 axis=mybir.AxisListType.X)
            wscr = pool.tile([4, D], F32, tag="wscr")
            nc.vector.scalar_tensor_tensor(
                wscr[:], P_w[:], 1.0, ebrow[:], op0=MULT, op1=MULT,
                accum_out=stg1[0:4, 32:33],
            )
            nc.vector.tensor_copy(stg1[0:4, 64:65], P_u[:, 255:256])
            zrt = pool.tile([4, 1], F32, tag="zrt")
            nc.vector.reciprocal(zrt[:], stg1[0:4, 0:1])

            stg1T = pool.tile([32, 96], F32, tag="stg1T")
            nc.vector.transpose(stg1T[:], stg1[:])
            Zrow = stg1T[0:1, 0:4]
            p1row = stg1T[0:1, 32:36]
            u255row = stg1T[0:1, 64:68]

            # A = 20*p1/Z - 10 ; Q = 20*u255
            zr4 = pool.tile([1, 4], F32, tag="zr4")
            nc.vector.reciprocal(zr4[:], Zrow)
            ta = pool.tile([1, 4], F32, tag="ta")
            nc.vector.tensor_mul(ta[:], p1row, zr4[:])
            A4 = pool.tile([1, 4], F32, tag="A4")
            nc.vector.tensor_scalar(A4[:], ta[:], 20.0, -10.0, op0=MULT, op1=ADD)
            Q4 = pool.tile([1, 4], F32, tag="Q4")
            nc.vector.tensor_scalar_mul(Q4[:], u255row, 20.0)
            B4 = pool.tile([1, 4], F32, tag="B4")
            nc.vector.tensor_mul(B4[:], Q4[:], zr4[:])

            # --- sequential chain over sigma_i = sigmoid(s_i); step 0 is a
            # compile-time constant (s_0 = -10), and s_{i+1} = A_i + B_i*sigma_i
            stg2 = pool.tile([32, 32], F32, tag="stg2")
            nc.vector.memset(stg2[:], 0.0)
            nc.vector.memset(stg2[0:1, 0:1], float(1.0 / (1.0 + np.exp(10.0))))
            s_t = pool.tile([1, 1], F32, tag="s1", name="s1")
            nc.vector.tensor_scalar(
                s_t[:], B4[0:1, 0:1], float(1.0 / (1.0 + np.exp(10.0))),
                A4[0:1, 0:1], op0=MULT, op1=ADD,
            )
            for i in range(1, 4):
                e_t = pool.tile([1, 1], F32, tag=f"e{i}", name=f"e{i}")
                nc.scalar.activation(e_t[:], s_t[:], EXP, scale=-1.0)
                t1 = pool.tile([1, 1], F32, tag=f"t1_{i}", name=f"t1_{i}")
                nc.vector.tensor_scalar_add(t1[:], e_t[:], 1.0)
                nc.vector.reciprocal(stg2[0:1, i:i + 1], t1[:])
                if i < 3:
                    s_n = pool.tile([1, 1], F32, tag=f"s{i + 1}",
                                    name=f"s{i+1}")
                    nc.vector.tensor_scalar(
                        s_n[:], stg2[0:1, i:i + 1],
                        B4[0:1, i:i + 1], A4[0:1, i:i + 1],
                        op0=MULT, op1=ADD,
                    )
                    s_t = s_n

            stg2T = pool.tile([32, 32], F32, tag="stg2T")
            nc.vector.transpose(stg2T[:], stg2[:])
            gaT = pool.tile([4, 1], F32, tag="gaT")
            nc.vector.tensor_mul(gaT[:], stg2T[0:4, 0:1], zrt[:])
            haT = pool.tile([4, 1], F32, tag="haT")
            nc.vector.tensor_sub(haT[:], zrt[:], gaT[:])

            # --- combine: out = haT*u + gaT*roll(u, 1)
            comb = pool.tile([4, D], F32, tag="comb")
            nc.scalar.activation(comb[:], P_u[:], COPY, scale=haT[:, 0:1])
            oout = pool.tile([4, D], F32, tag="oout")
            nc.vector.scalar_tensor_tensor(
                oout[:, 1:D], P_u[:, 0:D - 1], gaT[:, 0:1], comb[:, 1:D],
                op0=MULT, op1=ADD,
            )
            nc.vector.scalar_tensor_tensor(
                oout[:, 0:1], P_u[:, D - 1:D], gaT[:, 0:1], comb[:, 0:1],
                op0=MULT, op1=ADD,
            )
            nc.sync.dma_start(out.ap(), oout[:])

    nc.compile()
    return nc


def prep_inputs(a_emb, b_emb):
    a = np.ascontiguousarray(a_emb, dtype=np.float32)
    b = np.ascontiguousarray(b_emb, dtype=np.float32)
    arevT = a[:, ::-1].T                                     # [256, 4]
    arevT2 = np.empty((128, 8), np.float32)
    for c in range(2):
        arevT2[:, 4 * c:4 * c + 4] = arevT[128 * c:128 * (c + 1)]
    b2h = np.concatenate([b, b], axis=1).astype(np.float16)
    return {"arevT2": np.ascontiguousarray(arevT2),
            "b2h": np.ascontiguousarray(b2h)}


_NC_CACHE = {}


def run(a_emb, b_emb, trace=False):
    if "nc" not in _NC_CACHE:
        _NC_CACHE["nc"] = build_nc()
    nc = _NC_CACHE["nc"]
    in_map = prep_inputs(a_emb, b_emb)
    res = run_bass_kernel_spmd(
        nc, [in_map] * N_CORES, core_ids=list(range(N_CORES)), trace=trace
    )
    return np.asarray(res.results[0]["out"], dtype=np.float32), res


NUM_ENTRIES = 256 * 256 * 2


def _tables_match(W1, W2_sum, W2_carry):
    """Exact structural check of the deterministic one-hot tables."""
    try:
        W1 = np.asarray(W1)
        W2s = np.asarray(W2_sum)
        W2c = np.asarray(W2_carry)
        if (W1.shape != (514, NUM_ENTRIES) or W2s.shape != (NUM_ENTRIES, 256)
                or W2c.shape != (NUM_ENTRIES, 2)):
            return False
        idx = np.arange(NUM_ENTRIES)
        a = idx // 512
        b = (idx % 512) // 2
        c = idx % 2
        total = a + b + c
        # probed positions must be exactly 1 and |sum| must equal the count,
        # which (with the probes) pins every other entry to exactly 0
        if not (np.abs(W1).sum() == 3.0 * NUM_ENTRIES
                and (W1[a, idx] == 1.0).all()
                and (W1[256 + b, idx] == 1.0).all()
                and (W1[512 + c, idx] == 1.0).all()):
            return False
        if not (np.abs(W2s).sum() == float(NUM_ENTRIES)
                and (W2s[idx, total & 255] == 1.0).all()):
            return False
        if not (np.abs(W2c).sum() == float(NUM_ENTRIES)
                and (W2c[idx, (total >= 256).astype(np.int64)] == 1.0).all()):
            return False
        return True
    except Exception:
        return False


def _fallback_jax(a_emb, b_emb, W1, W2_sum, W2_carry):
    """Direct evaluation of the reference on the neuron devices via jax.
    Only reached if the tables are not the deterministic one-hot structure."""
    import jax
    import jax.numpy as jnp

    def step(carry, ab):
        a_i, b_i = ab
        x = jnp.concatenate([a_i, b_i, carry])
        scores = x @ jnp.asarray(W1)
        weights = jax.nn.softmax((scores - 2.5) * 10.0)
        return weights @ jnp.asarray(W2_carry), weights @ jnp.asarray(W2_sum)

    carry0 = jnp.zeros(2, dtype=jnp.float32).at[0].set(1.0)
    _, results = jax.lax.scan(
        step, carry0, (jnp.asarray(a_emb), jnp.asarray(b_emb))
    )
    return np.asarray(results, dtype=np.float32)


def kernel(a_emb, b_emb, W1, W2_sum, W2_carry):
    if not _tables_match(W1, W2_sum, W2_carry):
        return _fallback_jax(a_emb, b_emb, W1, W2_sum, W2_carry)
    o, _ = run(a_emb, b_emb, trace=False)
    return o

